# revision 14
# baseline (speedup 1.0000x reference)
"""Trainium2 Bass kernel for nn_DMPNNEncoder_64819646431794.

Full-input contract: kernel(**inputs) takes the complete (unsharded) numpy
inputs and returns the full [16384, 128] float32 output. Internally the
node axis N (columns of bond_n, rows of x/edge_attr/out) is sharded across
8 NeuronCores; the [D] message m is an AllReduce of per-shard partials.

Math (note: the reference layer loop never feeds h back, so only the last
layer's weights affect the output):
    xe   = atom_emb[x[:,0]] + atom_emb[x[:,1]]          # [N, D]
    he   = bond_emb[e[:,0]] + bond_emb[e[:,1]]          # [N, D]
    cat  = [xe, he]                                     # [N, 2D]
    colw = bond_n.sum(axis=0)                           # [N]   <- 1 GiB stream
    h_n  = relu(cat @ Wi.T + bi)                        # [N, D]
    m    = colw @ h_n                                   # [D]
    h    = relu(h_n + m @ Wm.T + bm)                    # [N, D]

Device-side layout: h_n is kept transposed [D, n] so biases and the
broadcast row m@Wm.T+bm are per-partition scalars. The embedding gather is
a one-hot matmul against host-folded tables aproj = atom_emb @ Wi[:,:D].T
and bproj = bond_emb @ Wi[:,D:].T.
"""

import sys

import numpy as np

if "/opt/trn_rl_repo" not in sys.path:
    sys.path.insert(0, "/opt/trn_rl_repo")

N_FULL = 16384
A_FULL = 16384
D = 128
P = 128
NCORES = 8
CHUNK = 512  # PSUM bank free width in f32
NUM_ATOM_TYPE = 120
NUM_BOND_TYPE = 6
DMA_SPLIT = 1  # 1 = full 8KB contiguous rows per descriptor


def build_program(
    a_rows, nc_shard, n_cores, use_collective=True, cc_shared=True, stream_reps=1
):
    """Build the SPMD Bass program (same program on every core)."""
    import concourse.bacc as bacc
    import concourse.mybir as mybir
    import concourse.tile as tile
    from concourse.masks import make_identity

    f32 = mybir.dt.float32
    f32r = mybir.dt.float32r
    Alu = mybir.AluOpType
    Act = mybir.ActivationFunctionType

    assert a_rows % P == 0 and nc_shard % CHUNK == 0
    RBLK = a_rows // P        # bond row blocks
    NBLK = nc_shard // P      # node blocks
    NCG = nc_shard // CHUNK   # 512-wide column groups
    BPC = CHUNK // P          # node blocks per column group

    nc = bacc.Bacc(
        "TRN2", target_bir_lowering=False, debug=False, num_devices=n_cores
    )

    bond = nc.declare_dram_parameter("bond", [a_rows, nc_shard], f32r, isOutput=False)
    onesw = nc.declare_dram_parameter("onesw", [P, P], f32r, isOutput=False)
    idx = nc.declare_dram_parameter("idx", [P, 4 * NBLK], f32, isOutput=False)
    aproj = nc.declare_dram_parameter("aproj", [NUM_ATOM_TYPE, D], f32, isOutput=False)
    bproj = nc.declare_dram_parameter("bproj", [NUM_BOND_TYPE, D], f32, isOutput=False)
    wmT = nc.declare_dram_parameter("wmT", [D, D], f32, isOutput=False)
    bi = nc.declare_dram_parameter("bi", [D, 1], f32, isOutput=False)
    bm = nc.declare_dram_parameter("bm", [D, 1], f32, isOutput=False)
    # [D, n] layout: written straight from the h_sb row-major relu pass;
    # the host-side unshard transposes back to [n, D].
    out = nc.declare_dram_parameter("out", [D, nc_shard], f32, isOutput=True)

    with tile.TileContext(nc) as tc:
        with (
            tc.tile_pool(name="const", bufs=1) as cpool,
            tc.tile_pool(name="bondp", bufs=12) as bpool,
            tc.tile_pool(name="small", bufs=3) as spool,
            tc.tile_pool(name="psA", bufs=2, space="PSUM") as psA,
            tc.tile_pool(name="psB", bufs=2, space="PSUM") as psB,
            tc.tile_pool(name="pscw", bufs=1, space="PSUM") as pscw,
            tc.tile_pool(name="dramp", bufs=1, space="DRAM") as dpool,
        ):
            # ---- constants -------------------------------------------------
            iota_f = cpool.tile([P, P], f32)
            nc.gpsimd.iota(
                iota_f[:],
                pattern=[[1, P]],
                base=0,
                channel_multiplier=0,
                allow_small_or_imprecise_dtypes=True,
            )
            ident = cpool.tile([P, P], f32)
            make_identity(nc, ident[:])
            # gpsimd memset can't write float32r (ISA check); DMA it in.
            ones_mat = cpool.tile([P, P], f32r)
            nc.scalar.dma_start(ones_mat[:], onesw[:])

            idx_sb = cpool.tile([P, 4 * NBLK], f32)
            nc.scalar.dma_start(idx_sb[:], idx[:])
            aproj_sb = cpool.tile([NUM_ATOM_TYPE, D], f32)
            nc.scalar.dma_start(aproj_sb[:], aproj[:])
            bproj_sb = cpool.tile([NUM_BOND_TYPE, D], f32)
            nc.scalar.dma_start(bproj_sb[:], bproj[:])
            wmT_sb = cpool.tile([D, D], f32)
            nc.scalar.dma_start(wmT_sb[:], wmT[:])
            bi_sb = cpool.tile([D, 1], f32)
            nc.scalar.dma_start(bi_sb[:], bi[:])
            bm_sb = cpool.tile([D, 1], f32)
            nc.scalar.dma_start(bm_sb[:], bm[:])

            # (A warmup AllReduce during the stream was tried and did NOT
            # shorten the real one — its cost is rendezvous/skew, not spin-up.)
            cc_space = "Shared" if (n_cores > 4 and cc_shared) else "Local"

            zeros_sb = cpool.tile([P, CHUNK], f32)
            nc.gpsimd.memset(zeros_sb[:], 0.0)

            h_sb = cpool.tile([P, nc_shard], f32)     # h_n transposed [D, n]
            mp = cpool.tile([P, NCG], f32)            # chained partial-m accum
            m_sb = cpool.tile([P, 1], f32)
            vec_sb = cpool.tile([P, 1], f32)

            # ---- phase 1: h_nT = relu(Wi @ catT + bi), via one-hot matmul --
            for c in range(NCG):
                psum_h = psA.tile([P, CHUNK], f32)
                for j in range(BPC):
                    b = c * BPC + j
                    ohx = spool.tile([P, NUM_ATOM_TYPE], f32)
                    nc.vector.tensor_scalar(
                        out=ohx[:],
                        in0=iota_f[:, :NUM_ATOM_TYPE],
                        scalar1=idx_sb[:, b : b + 1],
                        scalar2=None,
                        op0=Alu.is_equal,
                    )
                    ohx2 = spool.tile([P, NUM_ATOM_TYPE], f32)
                    nc.vector.scalar_tensor_tensor(
                        out=ohx2[:],
                        in0=iota_f[:, :NUM_ATOM_TYPE],
                        scalar=idx_sb[:, NBLK + b : NBLK + b + 1],
                        in1=ohx[:],
                        op0=Alu.is_equal,
                        op1=Alu.add,
                    )
                    ohxT_ps = psB.tile([NUM_ATOM_TYPE, P], f32, tag="scr")
                    nc.tensor.transpose(ohxT_ps[:], ohx2[:], ident[:])
                    ohxT = spool.tile([NUM_ATOM_TYPE, P], f32)
                    nc.scalar.copy(ohxT[:], ohxT_ps[:])

                    ohe = spool.tile([P, NUM_BOND_TYPE], f32)
                    nc.vector.tensor_scalar(
                        out=ohe[:],
                        in0=iota_f[:, :NUM_BOND_TYPE],
                        scalar1=idx_sb[:, 2 * NBLK + b : 2 * NBLK + b + 1],
                        scalar2=None,
                        op0=Alu.is_equal,
                    )
                    ohe2 = spool.tile([P, NUM_BOND_TYPE], f32)
                    nc.vector.scalar_tensor_tensor(
                        out=ohe2[:],
                        in0=iota_f[:, :NUM_BOND_TYPE],
                        scalar=idx_sb[:, 3 * NBLK + b : 3 * NBLK + b + 1],
                        in1=ohe[:],
                        op0=Alu.is_equal,
                        op1=Alu.add,
                    )
                    oheT_ps = psB.tile([NUM_BOND_TYPE, P], f32, tag="scr")
                    nc.tensor.transpose(oheT_ps[:], ohe2[:], ident[:])
                    oheT = spool.tile([NUM_BOND_TYPE, P], f32)
                    nc.scalar.copy(oheT[:], oheT_ps[:])

                    nc.tensor.matmul(
                        psum_h[:, j * P : (j + 1) * P],
                        lhsT=aproj_sb[:],
                        rhs=ohxT[:],
                        start=True,
                        stop=False,
                    )
                    nc.tensor.matmul(
                        psum_h[:, j * P : (j + 1) * P],
                        lhsT=bproj_sb[:],
                        rhs=oheT[:],
                        start=False,
                        stop=True,
                    )
                nc.scalar.activation(
                    h_sb[:, c * CHUNK : (c + 1) * CHUNK],
                    psum_h[:],
                    Act.Relu,
                    bias=bi_sb[:],
                )

            # ---- phase 2: colw = bond.sum(axis=0) via ones-matmul ----------
            cw_ps = [
                pscw.tile([P, CHUNK], f32, tag=f"cw{g}", name=f"cw{g}")
                for g in range(NCG)
            ]
            sub = nc_shard // DMA_SPLIT
            for _rep in range(stream_reps):  # >1 only for benchmarking
                for t_i in range(RBLK):
                    bt = bpool.tile([P, nc_shard], f32r, name="bt")
                    for s in range(DMA_SPLIT):
                        nc.sync.dma_start(
                            bt[:, s * sub : (s + 1) * sub],
                            bond[t_i * P : (t_i + 1) * P, s * sub : (s + 1) * sub],
                        )
                    for g in range(NCG):
                        # float32r: full-rate PE (1 cycle/row at free>=256)
                        # vs plain fp32's 4 cycles/row. Input bits are fp32;
                        # reduced-precision accumulate is fine for the 2e-2
                        # tolerance (colsum of ~16k uniform values).
                        nc.tensor.matmul(
                            cw_ps[g][:],
                            lhsT=ones_mat[:],
                            rhs=bt[:, g * CHUNK : (g + 1) * CHUNK],
                            start=(t_i == 0),
                            stop=(t_i == RBLK - 1),
                        )

            # ---- phase 3: m_partial = sum_n colw[n] * h_nT[:, n] -----------
            # cw_ps[g] rows are identical (the all-ones stationary matrix
            # broadcasts colw across partitions). Two-input DVE ops with a
            # PSUM operand kill the exec unit on this runtime, so ACT copies
            # PSUM->SBUF first and DVE stays SBUF-only.
            for g in range(NCG):
                cwsb = spool.tile([P, CHUNK], f32, name="cwsb")
                nc.scalar.copy(cwsb[:], cw_ps[g][:])
                ttmp = spool.tile([P, CHUNK], f32, name="ttmp")
                # NOTE: tensor_tensor_reduce dies with a runtime INTERNAL
                # error on this stack; keep separate mult + reduce.
                nc.vector.tensor_tensor(
                    ttmp[:],
                    h_sb[:, g * CHUNK : (g + 1) * CHUNK],
                    cwsb[:],
                    Alu.mult,
                )
                nc.vector.reduce_sum(
                    out=mp[:, g : g + 1],
                    in_=ttmp[:],
                    axis=mybir.AxisListType.X,
                )
            mtot = cpool.tile([P, 1], f32)
            nc.vector.reduce_sum(out=mtot[:], in_=mp[:], axis=mybir.AxisListType.X)

            # Fold Wm into the partial BEFORE the AllReduce (linearity):
            # AllReduce(Wm @ m_c) == Wm @ AllReduce(m_c); removes the matvec
            # from the post-collective critical path.
            mw_ps = psB.tile([P, 1], f32, tag="scr", name="mw_ps")
            nc.tensor.matmul(
                mw_ps[:], lhsT=wmT_sb[:], rhs=mtot[:], start=True, stop=True
            )
            vpre = cpool.tile([P, 1], f32)
            nc.scalar.copy(vpre[:], mw_ps[:])

            # ---- phase 4: AllReduce m over the 8 cores ---------------------
            cc_in = dpool.tile([P, 1], f32, name="cc_in")
            # Shared-scratchpad collective output is only supported for >4 cores.
            cc_out = dpool.tile([P, 1], f32, name="cc_out", addr_space=cc_space)
            nc.gpsimd.dma_start(cc_in[:], vpre[:])
            if use_collective:
                nc.gpsimd.collective_compute(
                    "AllReduce",
                    Alu.add,
                    replica_groups=[list(range(n_cores))],
                    ins=[cc_in[:].opt()],
                    outs=[cc_out[:].opt()],
                )
            else:
                nc.sync.dma_start(cc_out[:], cc_in[:])
            nc.gpsimd.dma_start(m_sb[:], cc_out[:])

            # ---- phase 5: h = relu(h_n + (Wm@m + bm)), [D, n] layout -------
            # m_sb already holds AllReduce(Wm @ m_partial) == Wm @ m.
            nc.vector.tensor_add(vec_sb[:], m_sb[:], bm_sb[:])
            for c in range(NCG):
                hf = spool.tile([P, CHUNK], f32, name="hf")
                if c % 2 == 0:
                    nc.scalar.activation(
                        hf[:],
                        h_sb[:, c * CHUNK : (c + 1) * CHUNK],
                        Act.Relu,
                        bias=vec_sb[:],
                    )
                else:
                    # relu(h+vec) == max(h+vec, 0) on DVE, halving the
                    # serial ACT chain on the post-collective tail.
                    nc.vector.scalar_tensor_tensor(
                        out=hf[:],
                        in0=h_sb[:, c * CHUNK : (c + 1) * CHUNK],
                        scalar=vec_sb[:, 0:1],
                        in1=zeros_sb[:],
                        op0=Alu.add,
                        op1=Alu.max,
                    )
                nc.sync.dma_start(out[:, c * CHUNK : (c + 1) * CHUNK], hf[:])

    nc.compile()
    return nc


def prep_inputs(inputs, a_rows, nc_shard, n_cores):
    """Host-side shard/prep: slice bond_n columns, reshape indices, fold the
    last layer's Wi into the embedding tables, transpose Wm."""
    x = np.asarray(inputs["x"]).astype(np.int64)
    ea = np.asarray(inputs["edge_attr"]).astype(np.int64)
    bond_n = np.asarray(inputs["bond_n"], dtype=np.float32)
    atom_emb = np.asarray(inputs["atom_emb"], dtype=np.float32)
    bond_emb = np.asarray(inputs["bond_emb"], dtype=np.float32)
    Wi = np.asarray(inputs["Wi_w"], dtype=np.float32)[-1]
    bi = np.asarray(inputs["Wi_b"], dtype=np.float32)[-1]
    Wm = np.asarray(inputs["Wm_w"], dtype=np.float32)[-1]
    bm = np.asarray(inputs["Wm_b"], dtype=np.float32)[-1]

    aproj = np.ascontiguousarray(atom_emb @ Wi[:, :D].T, dtype=np.float32)
    bproj = np.ascontiguousarray(bond_emb @ Wi[:, D:].T, dtype=np.float32)
    wmT = np.ascontiguousarray(Wm.T, dtype=np.float32)
    bi2 = np.ascontiguousarray(bi.reshape(D, 1), dtype=np.float32)
    bm2 = np.ascontiguousarray(bm.reshape(D, 1), dtype=np.float32)

    nblk = nc_shard // P
    in_maps = []
    for k in range(n_cores):
        cols = slice(k * nc_shard, (k + 1) * nc_shard)
        idx = np.concatenate(
            [
                x[cols, 0].reshape(nblk, P).T,
                x[cols, 1].reshape(nblk, P).T,
                ea[cols, 0].reshape(nblk, P).T,
                ea[cols, 1].reshape(nblk, P).T,
            ],
            axis=1,
        ).astype(np.float32)
        in_maps.append(
            {
                "bond": np.ascontiguousarray(bond_n[:a_rows, cols]),
                "onesw": np.ones((P, P), dtype=np.float32),
                "idx": np.ascontiguousarray(idx),
                "aproj": aproj,
                "bproj": bproj,
                "wmT": wmT,
                "bi": bi2,
                "bm": bm2,
            }
        )
    return in_maps


_PROGRAM_CACHE = {}


def _get_program(a_rows, nc_shard, n_cores, **kw):
    key = (a_rows, nc_shard, n_cores, tuple(sorted(kw.items())))
    if key not in _PROGRAM_CACHE:
        _PROGRAM_CACHE[key] = build_program(a_rows, nc_shard, n_cores, **kw)
    return _PROGRAM_CACHE[key]


def run(inputs, trace=False, **kw):
    from concourse.bass_utils import run_bass_kernel_spmd

    n = np.asarray(inputs["bond_n"]).shape[1]
    a_rows = np.asarray(inputs["bond_n"]).shape[0]
    nc_shard = n // NCORES
    nc = _get_program(a_rows, nc_shard, NCORES, **kw)
    in_maps = prep_inputs(inputs, a_rows, nc_shard, NCORES)
    res = run_bass_kernel_spmd(nc, in_maps, list(range(NCORES)), trace=trace)
    h = np.concatenate(
        [np.ascontiguousarray(res.results[k]["out"].T) for k in range(NCORES)],
        axis=0,
    )
    return h, res


def kernel(**inputs) -> np.ndarray:
    h, _ = run(inputs, trace=False)
    return h



# revision 16
# speedup vs baseline: 1.1764x; 1.1764x over previous
"""Trainium2 Bass kernel for nn_DMPNNEncoder_64819646431794.

Full-input contract: kernel(**inputs) takes the complete (unsharded) numpy
inputs and returns the full [16384, 128] float32 output. Internally the
node axis N (columns of bond_n, rows of x/edge_attr/out) is sharded across
8 NeuronCores; the [D] message m is an AllReduce of per-shard partials.

Math (note: the reference layer loop never feeds h back, so only the last
layer's weights affect the output):
    xe   = atom_emb[x[:,0]] + atom_emb[x[:,1]]          # [N, D]
    he   = bond_emb[e[:,0]] + bond_emb[e[:,1]]          # [N, D]
    cat  = [xe, he]                                     # [N, 2D]
    colw = bond_n.sum(axis=0)                           # [N]   <- 1 GiB stream
    h_n  = relu(cat @ Wi.T + bi)                        # [N, D]
    m    = colw @ h_n                                   # [D]
    h    = relu(h_n + m @ Wm.T + bm)                    # [N, D]

Device-side layout: h_n is kept transposed [D, n] so biases and the
broadcast row m@Wm.T+bm are per-partition scalars. The embedding gather is
a one-hot matmul against host-folded tables aproj = atom_emb @ Wi[:,:D].T
and bproj = bond_emb @ Wi[:,D:].T.
"""

import sys

import numpy as np

if "/opt/trn_rl_repo" not in sys.path:
    sys.path.insert(0, "/opt/trn_rl_repo")

N_FULL = 16384
A_FULL = 16384
D = 128
P = 128
NCORES = 8
CHUNK = 512  # PSUM bank free width in f32
NUM_ATOM_TYPE = 120
NUM_BOND_TYPE = 6
DMA_SPLIT = 1  # 1 = full 8KB contiguous rows per descriptor


def build_program(
    a_rows, nc_shard, n_cores, use_collective=True, cc_shared=True, stream_reps=1
):
    """Build the SPMD Bass program (same program on every core)."""
    import concourse.bacc as bacc
    import concourse.mybir as mybir
    import concourse.tile as tile
    from concourse.masks import make_identity

    f32 = mybir.dt.float32
    f32r = mybir.dt.float32r
    Alu = mybir.AluOpType
    Act = mybir.ActivationFunctionType

    assert a_rows % P == 0 and nc_shard % CHUNK == 0
    RBLK = a_rows // P        # bond row blocks
    NBLK = nc_shard // P      # node blocks
    NCG = nc_shard // CHUNK   # 512-wide column groups
    BPC = CHUNK // P          # node blocks per column group

    nc = bacc.Bacc(
        "TRN2", target_bir_lowering=False, debug=False, num_devices=n_cores
    )

    bond = nc.declare_dram_parameter("bond", [a_rows, nc_shard], f32r, isOutput=False)
    onesw = nc.declare_dram_parameter("onesw", [P, P], f32r, isOutput=False)
    idx = nc.declare_dram_parameter("idx", [P, 4 * NBLK], f32, isOutput=False)
    aproj = nc.declare_dram_parameter("aproj", [NUM_ATOM_TYPE, D], f32, isOutput=False)
    bproj = nc.declare_dram_parameter("bproj", [NUM_BOND_TYPE, D], f32, isOutput=False)
    wmT = nc.declare_dram_parameter("wmT", [D, D], f32, isOutput=False)
    bi = nc.declare_dram_parameter("bi", [D, 1], f32, isOutput=False)
    bm = nc.declare_dram_parameter("bm", [D, 1], f32, isOutput=False)
    # [D, n] layout: written straight from the h_sb row-major relu pass;
    # the host-side unshard transposes back to [n, D].
    out = nc.declare_dram_parameter("out", [D, nc_shard], f32, isOutput=True)

    with tile.TileContext(nc) as tc:
        with (
            tc.tile_pool(name="const", bufs=1) as cpool,
            tc.tile_pool(name="bondp", bufs=12) as bpool,
            tc.tile_pool(name="small", bufs=3) as spool,
            tc.tile_pool(name="psA", bufs=2, space="PSUM") as psA,
            tc.tile_pool(name="psB", bufs=2, space="PSUM") as psB,
            tc.tile_pool(name="pscw", bufs=1, space="PSUM") as pscw,
            tc.tile_pool(name="dramp", bufs=1, space="DRAM") as dpool,
        ):
            # ---- constants -------------------------------------------------
            iota_f = cpool.tile([P, P], f32)
            nc.gpsimd.iota(
                iota_f[:],
                pattern=[[1, P]],
                base=0,
                channel_multiplier=0,
                allow_small_or_imprecise_dtypes=True,
            )
            ident = cpool.tile([P, P], f32)
            make_identity(nc, ident[:])
            # gpsimd memset can't write float32r (ISA check); DMA it in.
            ones_mat = cpool.tile([P, P], f32r)
            nc.scalar.dma_start(ones_mat[:], onesw[:])

            idx_sb = cpool.tile([P, 4 * NBLK], f32)
            nc.scalar.dma_start(idx_sb[:], idx[:])
            aproj_sb = cpool.tile([NUM_ATOM_TYPE, D], f32)
            nc.scalar.dma_start(aproj_sb[:], aproj[:])
            bproj_sb = cpool.tile([NUM_BOND_TYPE, D], f32)
            nc.scalar.dma_start(bproj_sb[:], bproj[:])
            wmT_sb = cpool.tile([D, D], f32)
            nc.scalar.dma_start(wmT_sb[:], wmT[:])
            bi_sb = cpool.tile([D, 1], f32)
            nc.scalar.dma_start(bi_sb[:], bi[:])
            bm_sb = cpool.tile([D, 1], f32)
            nc.scalar.dma_start(bm_sb[:], bm[:])

            # (A warmup AllReduce during the stream was tried and did NOT
            # shorten the real one — its cost is rendezvous/skew, not spin-up.)
            cc_space = "Shared" if (n_cores > 4 and cc_shared) else "Local"

            zeros_sb = cpool.tile([P, CHUNK], f32)
            nc.gpsimd.memset(zeros_sb[:], 0.0)

            h_sb = cpool.tile([P, nc_shard], f32)     # h_n transposed [D, n]
            mp = cpool.tile([P, NCG], f32)            # chained partial-m accum
            m_sb = cpool.tile([P, 1], f32)
            vec_sb = cpool.tile([P, 1], f32)

            # ---- phase 1: h_nT = relu(Wi @ catT + bi), via one-hot matmul --
            for c in range(NCG):
                psum_h = psA.tile([P, CHUNK], f32)
                for j in range(BPC):
                    b = c * BPC + j
                    ohx = spool.tile([P, NUM_ATOM_TYPE], f32)
                    nc.vector.tensor_scalar(
                        out=ohx[:],
                        in0=iota_f[:, :NUM_ATOM_TYPE],
                        scalar1=idx_sb[:, b : b + 1],
                        scalar2=None,
                        op0=Alu.is_equal,
                    )
                    ohx2 = spool.tile([P, NUM_ATOM_TYPE], f32)
                    nc.vector.scalar_tensor_tensor(
                        out=ohx2[:],
                        in0=iota_f[:, :NUM_ATOM_TYPE],
                        scalar=idx_sb[:, NBLK + b : NBLK + b + 1],
                        in1=ohx[:],
                        op0=Alu.is_equal,
                        op1=Alu.add,
                    )
                    ohxT_ps = psB.tile([NUM_ATOM_TYPE, P], f32, tag="scr")
                    nc.tensor.transpose(ohxT_ps[:], ohx2[:], ident[:])
                    ohxT = spool.tile([NUM_ATOM_TYPE, P], f32)
                    nc.scalar.copy(ohxT[:], ohxT_ps[:])

                    ohe = spool.tile([P, NUM_BOND_TYPE], f32)
                    nc.vector.tensor_scalar(
                        out=ohe[:],
                        in0=iota_f[:, :NUM_BOND_TYPE],
                        scalar1=idx_sb[:, 2 * NBLK + b : 2 * NBLK + b + 1],
                        scalar2=None,
                        op0=Alu.is_equal,
                    )
                    ohe2 = spool.tile([P, NUM_BOND_TYPE], f32)
                    nc.vector.scalar_tensor_tensor(
                        out=ohe2[:],
                        in0=iota_f[:, :NUM_BOND_TYPE],
                        scalar=idx_sb[:, 3 * NBLK + b : 3 * NBLK + b + 1],
                        in1=ohe[:],
                        op0=Alu.is_equal,
                        op1=Alu.add,
                    )
                    oheT_ps = psB.tile([NUM_BOND_TYPE, P], f32, tag="scr")
                    nc.tensor.transpose(oheT_ps[:], ohe2[:], ident[:])
                    oheT = spool.tile([NUM_BOND_TYPE, P], f32)
                    nc.scalar.copy(oheT[:], oheT_ps[:])

                    nc.tensor.matmul(
                        psum_h[:, j * P : (j + 1) * P],
                        lhsT=aproj_sb[:],
                        rhs=ohxT[:],
                        start=True,
                        stop=False,
                    )
                    nc.tensor.matmul(
                        psum_h[:, j * P : (j + 1) * P],
                        lhsT=bproj_sb[:],
                        rhs=oheT[:],
                        start=False,
                        stop=True,
                    )
                nc.scalar.activation(
                    h_sb[:, c * CHUNK : (c + 1) * CHUNK],
                    psum_h[:],
                    Act.Relu,
                    bias=bi_sb[:],
                )

            # ---- phase 2: colw = bond.sum(axis=0) via ones-matmul ----------
            cw_ps = [
                pscw.tile([P, CHUNK], f32, tag=f"cw{g}", name=f"cw{g}")
                for g in range(NCG)
            ]
            sub = nc_shard // DMA_SPLIT
            for _rep in range(stream_reps):  # >1 only for benchmarking
                for t_i in range(RBLK):
                    bt = bpool.tile([P, nc_shard], f32r, name="bt")
                    # Alternate issue queues: two HWDGE queues fetch
                    # descriptors independently.
                    eng = nc.sync if t_i % 2 == 0 else nc.scalar
                    for s in range(DMA_SPLIT):
                        eng.dma_start(
                            bt[:, s * sub : (s + 1) * sub],
                            bond[t_i * P : (t_i + 1) * P, s * sub : (s + 1) * sub],
                        )
                    for g in range(NCG):
                        # float32r: full-rate PE (1 cycle/row at free>=256)
                        # vs plain fp32's 4 cycles/row. Input bits are fp32;
                        # reduced-precision accumulate is fine for the 2e-2
                        # tolerance (colsum of ~16k uniform values).
                        nc.tensor.matmul(
                            cw_ps[g][:],
                            lhsT=ones_mat[:],
                            rhs=bt[:, g * CHUNK : (g + 1) * CHUNK],
                            start=(t_i == 0),
                            stop=(t_i == RBLK - 1),
                        )

            # ---- phase 3: m_partial = sum_n colw[n] * h_nT[:, n] -----------
            # cw_ps[g] rows are identical (the all-ones stationary matrix
            # broadcasts colw across partitions). Two-input DVE ops with a
            # PSUM operand kill the exec unit on this runtime, so ACT copies
            # PSUM->SBUF first and DVE stays SBUF-only.
            for g in range(NCG):
                cwsb = spool.tile([P, CHUNK], f32, name="cwsb")
                nc.scalar.copy(cwsb[:], cw_ps[g][:])
                ttmp = spool.tile([P, CHUNK], f32, name="ttmp")
                # NOTE: tensor_tensor_reduce dies with a runtime INTERNAL
                # error on this stack; keep separate mult + reduce. Split
                # the groups across DVE and GpSimd to halve the serial chain.
                veng = nc.vector if g % 2 == 0 else nc.gpsimd
                veng.tensor_tensor(
                    ttmp[:],
                    h_sb[:, g * CHUNK : (g + 1) * CHUNK],
                    cwsb[:],
                    Alu.mult,
                )
                # gpsimd cannot reduce along the free axis; DVE does all 4.
                nc.vector.reduce_sum(
                    out=mp[:, g : g + 1],
                    in_=ttmp[:],
                    axis=mybir.AxisListType.X,
                )
            mtot = cpool.tile([P, 1], f32)
            nc.vector.reduce_sum(out=mtot[:], in_=mp[:], axis=mybir.AxisListType.X)

            # Fold Wm into the partial BEFORE the AllReduce (linearity):
            # AllReduce(Wm @ m_c) == Wm @ AllReduce(m_c); removes the matvec
            # from the post-collective critical path.
            mw_ps = psB.tile([P, 1], f32, tag="scr", name="mw_ps")
            nc.tensor.matmul(
                mw_ps[:], lhsT=wmT_sb[:], rhs=mtot[:], start=True, stop=True
            )
            vpre = cpool.tile([P, 1], f32)
            nc.scalar.copy(vpre[:], mw_ps[:])

            # ---- phase 4: AllReduce m over the 8 cores ---------------------
            cc_in = dpool.tile([P, 1], f32, name="cc_in")
            # Shared-scratchpad collective output is only supported for >4 cores.
            cc_out = dpool.tile([P, 1], f32, name="cc_out", addr_space=cc_space)
            nc.gpsimd.dma_start(cc_in[:], vpre[:])
            if use_collective:
                nc.gpsimd.collective_compute(
                    "AllReduce",
                    Alu.add,
                    replica_groups=[list(range(n_cores))],
                    ins=[cc_in[:].opt()],
                    outs=[cc_out[:].opt()],
                )
            else:
                nc.sync.dma_start(cc_out[:], cc_in[:])
            nc.gpsimd.dma_start(m_sb[:], cc_out[:])

            # ---- phase 5: h = relu(h_n + (Wm@m + bm)), [D, n] layout -------
            # m_sb already holds AllReduce(Wm @ m_partial) == Wm @ m.
            nc.vector.tensor_add(vec_sb[:], m_sb[:], bm_sb[:])
            outbuf = cpool.tile([P, nc_shard], f32)
            for c in range(NCG):
                dst = outbuf[:, c * CHUNK : (c + 1) * CHUNK]
                if c % 2 == 0:
                    nc.scalar.activation(
                        dst,
                        h_sb[:, c * CHUNK : (c + 1) * CHUNK],
                        Act.Relu,
                        bias=vec_sb[:],
                    )
                else:
                    # relu(h+vec) == max(h+vec, 0) on DVE, halving the
                    # serial ACT chain on the post-collective tail.
                    nc.vector.scalar_tensor_tensor(
                        out=dst,
                        in0=h_sb[:, c * CHUNK : (c + 1) * CHUNK],
                        scalar=vec_sb[:, 0:1],
                        in1=zeros_sb[:],
                        op0=Alu.add,
                        op1=Alu.max,
                    )
                nc.sync.dma_start(out[:, c * CHUNK : (c + 1) * CHUNK], dst)

    nc.compile()
    return nc


def prep_inputs(inputs, a_rows, nc_shard, n_cores):
    """Host-side shard/prep: slice bond_n columns, reshape indices, fold the
    last layer's Wi into the embedding tables, transpose Wm."""
    x = np.asarray(inputs["x"]).astype(np.int64)
    ea = np.asarray(inputs["edge_attr"]).astype(np.int64)
    bond_n = np.asarray(inputs["bond_n"], dtype=np.float32)
    atom_emb = np.asarray(inputs["atom_emb"], dtype=np.float32)
    bond_emb = np.asarray(inputs["bond_emb"], dtype=np.float32)
    Wi = np.asarray(inputs["Wi_w"], dtype=np.float32)[-1]
    bi = np.asarray(inputs["Wi_b"], dtype=np.float32)[-1]
    Wm = np.asarray(inputs["Wm_w"], dtype=np.float32)[-1]
    bm = np.asarray(inputs["Wm_b"], dtype=np.float32)[-1]

    aproj = np.ascontiguousarray(atom_emb @ Wi[:, :D].T, dtype=np.float32)
    bproj = np.ascontiguousarray(bond_emb @ Wi[:, D:].T, dtype=np.float32)
    wmT = np.ascontiguousarray(Wm.T, dtype=np.float32)
    bi2 = np.ascontiguousarray(bi.reshape(D, 1), dtype=np.float32)
    bm2 = np.ascontiguousarray(bm.reshape(D, 1), dtype=np.float32)

    nblk = nc_shard // P
    in_maps = []
    for k in range(n_cores):
        cols = slice(k * nc_shard, (k + 1) * nc_shard)
        idx = np.concatenate(
            [
                x[cols, 0].reshape(nblk, P).T,
                x[cols, 1].reshape(nblk, P).T,
                ea[cols, 0].reshape(nblk, P).T,
                ea[cols, 1].reshape(nblk, P).T,
            ],
            axis=1,
        ).astype(np.float32)
        in_maps.append(
            {
                "bond": np.ascontiguousarray(bond_n[:a_rows, cols]),
                "onesw": np.ones((P, P), dtype=np.float32),
                "idx": np.ascontiguousarray(idx),
                "aproj": aproj,
                "bproj": bproj,
                "wmT": wmT,
                "bi": bi2,
                "bm": bm2,
            }
        )
    return in_maps


_PROGRAM_CACHE = {}


def _get_program(a_rows, nc_shard, n_cores, **kw):
    key = (a_rows, nc_shard, n_cores, tuple(sorted(kw.items())))
    if key not in _PROGRAM_CACHE:
        _PROGRAM_CACHE[key] = build_program(a_rows, nc_shard, n_cores, **kw)
    return _PROGRAM_CACHE[key]


def run(inputs, trace=False, **kw):
    from concourse.bass_utils import run_bass_kernel_spmd

    n = np.asarray(inputs["bond_n"]).shape[1]
    a_rows = np.asarray(inputs["bond_n"]).shape[0]
    nc_shard = n // NCORES
    nc = _get_program(a_rows, nc_shard, NCORES, **kw)
    in_maps = prep_inputs(inputs, a_rows, nc_shard, NCORES)
    res = run_bass_kernel_spmd(nc, in_maps, list(range(NCORES)), trace=trace)
    h = np.concatenate(
        [np.ascontiguousarray(res.results[k]["out"].T) for k in range(NCORES)],
        axis=0,
    )
    return h, res


def kernel(**inputs) -> np.ndarray:
    h, _ = run(inputs, trace=False)
    return h

